# revision 16
# baseline (speedup 1.0000x reference)
"""GQA attention kernel for 8 TRN2 NeuronCores.

Sharding: core c handles batch b=c//2 and head-half h=c%2 (8 q heads, 2 kv
heads per core).  Projections are column-parallel (q/k/v) and row-parallel
(out_proj); the host sums the two partial outputs per batch (no on-device
collectives).

The reference "rope" degenerates to an elementwise scale Y *= C with
C[t,j] = cos(t*inv[j%64]) + sin(t*inv[j%64]), folded into the q/k PSUM
eviction.  Softmax is computed without max-subtraction (scores are O(10),
exp is safe in f32): scores are built transposed (ki on partitions, qi on
free) so exp lands directly in the layout the y-matmul needs; the row sums
are accumulated with an all-ones lhsT matmul which also broadcasts them
across all 128 partitions for the final divide.
"""

import sys

if '/opt/trn_rl_repo' not in sys.path:
    sys.path.insert(0, '/opt/trn_rl_repo')

import numpy as np

N_EMBD = 2048
HD = 128          # head dim
T = 1024          # seq len
B = 4             # batch
NK = 16           # contraction tiles over n_embd
P = 128
F32 = None        # filled after mybir import
SCALE = 1.0 / np.sqrt(HD)

_RUNNER = None
_NC = None


def _build_runner():
    from concourse import bacc, tile, mybir
    from concourse.bass_utils import run_bass_kernel_spmd

    f32 = mybir.dt.float32
    f32r = mybir.dt.float32r
    AF = mybir.ActivationFunctionType
    ALU = mybir.AluOpType

    nc = bacc.Bacc("TRN2", target_bir_lowering=False, debug=False, num_devices=8)

    xp = nc.dram_tensor("xp", [P, NK * T], f32r, kind="ExternalInput").ap()
    wq = nc.dram_tensor("wq", [8, P, 2048], f32r, kind="ExternalInput").ap()
    wk = nc.dram_tensor("wk", [P, NK * 256], f32r, kind="ExternalInput").ap()
    wv = nc.dram_tensor("wv", [P, NK * 256], f32r, kind="ExternalInput").ap()
    wo = nc.dram_tensor("wo", [16, P, 1024], f32r, kind="ExternalInput").ap()
    ct = nc.dram_tensor("ct", [P, T], f32, kind="ExternalInput").ap()
    mk = nc.dram_tensor("mk", [P, 2 * 256], f32, kind="ExternalInput").ap()
    bqd = nc.dram_tensor("bqd", [P, 8], f32, kind="ExternalInput").ap()
    bkd = nc.dram_tensor("bkd", [P, 2], f32, kind="ExternalInput").ap()
    bvd = nc.dram_tensor("bvd", [P, 256], f32, kind="ExternalInput").ap()
    oned = nc.dram_tensor("oned", [P, P], f32r, kind="ExternalInput").ap()
    out = nc.dram_tensor("out", [2048, T], f32, kind="ExternalOutput").ap()

    with tile.TileContext(nc) as tc:
        with (
            tc.tile_pool(name="const", bufs=1) as cpool,
            tc.tile_pool(name="qkv", bufs=1) as qkvpool,
        ):
            ct_sb = cpool.tile([P, T], f32, tag="ct")
            mk_sb = cpool.tile([P, 2 * 256], f32, tag="mk")
            bq_sb = cpool.tile([P, 8], f32, tag="bq")
            bk_sb = cpool.tile([P, 2], f32, tag="bk")
            bv_sb = cpool.tile([P, 256], f32, tag="bv")
            ones_sb = cpool.tile([P, P], f32r, tag="ones")

            qT = [qkvpool.tile([P, T], f32r, tag=f"qT{g}", name=f"qT{g}") for g in range(8)]
            kT = [qkvpool.tile([P, T], f32r, tag=f"kT{m}", name=f"kT{m}") for m in range(2)]
            vsb = [qkvpool.tile([P, 256], f32r, tag=f"v{tt}", name=f"v{tt}") for tt in range(8)]

            # ---------------- phase 1: projections ----------------
            with (
                tc.tile_pool(name="xt", bufs=16) as xpool,
                tc.tile_pool(name="wkv", bufs=32) as wkvpool,
                tc.tile_pool(name="wqs", bufs=2) as wqpool,
                tc.tile_pool(name="pp", bufs=8, space="PSUM") as pppool,
            ):
                x_sb = []
                wk_sb = []
                wv_sb = []
                for kc in range(NK):
                    xt = xpool.tile([P, T], f32r, tag="x")
                    nc.sync.dma_start(xt[:], xp[:, kc * T:(kc + 1) * T])
                    x_sb.append(xt)
                    wkt = wkvpool.tile([P, 256], f32r, tag="wk")
                    nc.sync.dma_start(wkt[:], wk[:, kc * 256:(kc + 1) * 256])
                    wk_sb.append(wkt)
                    wvt = wkvpool.tile([P, 256], f32r, tag="wv")
                    wv_sb.append(wvt)
                for kc in range(NK):
                    nc.sync.dma_start(wv_sb[kc][:], wv[:, kc * 256:(kc + 1) * 256])
                    if kc == 8:
                        nc.sync.dma_start(bk_sb[:], bkd[:])
                        nc.sync.dma_start(bv_sb[:], bvd[:])
                        nc.sync.dma_start(bq_sb[:], bqd[:])
                        nc.sync.dma_start(ct_sb[:], ct[:])
                        nc.sync.dma_start(ones_sb[:], oned[:])
                        nc.sync.dma_start(mk_sb[:], mk[:])

                # k projection: kT[m] (d on partitions, t free)
                for m in range(2):
                    for n in range(2):
                        ps = pppool.tile([P, 512], f32, tag="pp")
                        for kc in range(NK):
                            nc.tensor.matmul(
                                ps[:],
                                lhsT=wk_sb[kc][:, 128 * m:128 * m + 128],
                                rhs=x_sb[kc][:, 512 * n:512 * n + 512],
                                start=(kc == 0), stop=(kc == NK - 1),
                            )
                        nc.vector.scalar_tensor_tensor(
                            out=kT[m][:, 512 * n:512 * n + 512],
                            in0=ps[:], scalar=bk_sb[:, m:m + 1],
                            in1=ct_sb[:, 512 * n:512 * n + 512],
                            op0=ALU.add, op1=ALU.mult,
                        )

                # v projection: v (t on partitions, kv-dim free)
                for tt in range(8):
                    ps = pppool.tile([P, 256], f32, tag="pp")
                    for kc in range(NK):
                        nc.tensor.matmul(
                            ps[:],
                            lhsT=x_sb[kc][:, 128 * tt:128 * tt + 128],
                            rhs=wv_sb[kc][:],
                            start=(kc == 0), stop=(kc == NK - 1),
                        )
                    nc.vector.tensor_add(vsb[tt][:], ps[:], bv_sb[:])

                # q projection: qT[g] (d on partitions, t free)
                for g in range(8):
                    wqt = wqpool.tile([P, 2048], f32r, tag="wq")
                    nc.sync.dma_start(wqt[:], wq[g])
                    for n in range(2):
                        ps = pppool.tile([P, 512], f32, tag="pp")
                        for kc in range(NK):
                            nc.tensor.matmul(
                                ps[:],
                                lhsT=wqt[:, 128 * kc:128 * kc + 128],
                                rhs=x_sb[kc][:, 512 * n:512 * n + 512],
                                start=(kc == 0), stop=(kc == NK - 1),
                            )
                        nc.vector.scalar_tensor_tensor(
                            out=qT[g][:, 512 * n:512 * n + 512],
                            in0=ps[:], scalar=bq_sb[:, g:g + 1],
                            in1=ct_sb[:, 512 * n:512 * n + 512],
                            op0=ALU.add, op1=ALU.mult,
                        )

            # ---------------- phase 2+3: attention + out-proj ----------------
            with (
                tc.tile_pool(name="yT", bufs=1) as ypool,
                tc.tile_pool(name="exp", bufs=4) as epool,
                tc.tile_pool(name="rcp", bufs=2) as rpool,
                tc.tile_pool(name="wos", bufs=3) as wopool,
                tc.tile_pool(name="ost", bufs=4) as ostpool,
                tc.tile_pool(name="ps_s", bufs=2, space="PSUM") as spsum,
                tc.tile_pool(name="ps_y", bufs=1, space="PSUM") as ypsum,
                tc.tile_pool(name="ps_n", bufs=1, space="PSUM") as npsum,
                tc.tile_pool(name="ps_o", bufs=2, space="PSUM") as opsum,
            ):
                yT = [ypool.tile([P, T], f32r, tag=f"yT{g}", name=f"yT{g}") for g in range(8)]

                for c in range(4):
                    for g in range(8):
                        kg = g // 4
                        ps_y = ypsum.tile([P, 256], f32, tag="y")
                        ps_n = npsum.tile([P, 256], f32, tag="n")
                        R = 2 * c + 2
                        q_sl = qT[g][:, 256 * c:256 * c + 256]
                        # score blocks packed in groups of <=4 per 2-bank psum
                        # tile, one wide exp per pack
                        e_packs = []
                        for p0 in range(0, R, 4):
                            W = min(4, R - p0) * 256
                            ps_s = spsum.tile([P, 1024], f32, tag="s")
                            for j in range((W + 255) // 256):
                                nc.tensor.matmul(
                                    ps_s[:, 256 * j:256 * j + 256],
                                    lhsT=kT[kg][:, 128 * (p0 + j):128 * (p0 + j) + 128],
                                    rhs=q_sl,
                                    start=True, stop=True,
                                )
                            e = epool.tile([P, 1024], f32r, tag="e")
                            nc.scalar.activation(
                                e[:, 0:W], ps_s[:, 0:W], AF.Exp, scale=SCALE)
                            e_packs.append(e)
                        for rr in range(R):
                            e_sl = e_packs[rr // 4][:, 256 * (rr % 4):256 * (rr % 4) + 256]
                            if rr >= 2 * c:
                                i = rr - 2 * c
                                nc.vector.tensor_mul(
                                    e_sl, e_sl, mk_sb[:, 256 * i:256 * i + 256])
                            nc.tensor.matmul(
                                ps_y[:],
                                lhsT=vsb[rr][:, 128 * kg:128 * kg + 128],
                                rhs=e_sl,
                                start=(rr == 0), stop=(rr == R - 1),
                            )
                            nc.tensor.matmul(
                                ps_n[:],
                                lhsT=ones_sb[:],
                                rhs=e_sl,
                                start=(rr == 0), stop=(rr == R - 1),
                            )
                        rc = rpool.tile([P, 256], f32, tag="rc")
                        nc.vector.reciprocal(rc[:], ps_n[:])
                        nc.vector.tensor_mul(
                            yT[g][:, 256 * c:256 * c + 256], ps_y[:], rc[:])

                # out projection: outT (o on partitions, t free), partial sum
                for m in range(16):
                    wot = wopool.tile([P, 1024], f32r, tag="wo")
                    nc.sync.dma_start(wot[:], wo[m])
                    for n in range(2):
                        ps = opsum.tile([P, 512], f32, tag="o")
                        for kj in range(8):
                            nc.tensor.matmul(
                                ps[:],
                                lhsT=wot[:, 128 * kj:128 * kj + 128],
                                rhs=yT[kj][:, 512 * n:512 * n + 512],
                                start=(kj == 0), stop=(kj == 7),
                            )
                        ot = ostpool.tile([P, 512], f32, tag="ost")
                        nc.scalar.copy(ot[:], ps[:])
                        nc.sync.dma_start(
                            out[128 * m:128 * m + 128, 512 * n:512 * n + 512], ot[:])

    nc.compile()
    global _NC
    _NC = nc

    def run(in_maps, **kw):
        return run_bass_kernel_spmd(nc, in_maps, core_ids=list(range(8)), **kw)

    return run


def _host_prep(x, Wq, bq, Wk, bk, Wv, bv, Wo, bo):
    """Build the 8 per-core input maps."""
    inv = 10000.0 ** (-2.0 * np.arange(HD // 2) / HD)
    theta = np.arange(T)[:, None] * inv[None, :]
    C = np.concatenate([np.cos(theta) + np.sin(theta)] * 2, 1).astype(np.float32)
    ct = np.ascontiguousarray(C.T)                              # (128, 1024)

    mask = np.zeros((P, 2 * 256), np.float32)
    jj = np.arange(256)
    pp = np.arange(P)[:, None]
    for i in range(2):
        mask[:, 256 * i:256 * (i + 1)] = (jj[None, :] >= 128 * i + pp)

    in_maps = []
    for c in range(8):
        b, h = c // 2, c % 2
        xb = x[b]                                               # (t, 2048)
        # x_pre[p, kc*T + t] = x[b, t, 128*kc + p]
        xpre = np.ascontiguousarray(
            xb.reshape(T, NK, P).transpose(2, 1, 0).reshape(P, NK * T))
        Wq_l = Wq[1024 * h:1024 * h + 1024]
        # wq_pre[m, p, kc*128 + j] = Wq_l[128m+j, 128kc+p]
        wqpre = np.ascontiguousarray(
            Wq_l.reshape(8, P, NK, P).transpose(0, 3, 2, 1).reshape(8, P, 2048))
        Wk_l = Wk[256 * h:256 * h + 256]
        # wk_pre[p, kc*256 + j] = Wk_l[j, 128kc+p]
        wkpre = np.ascontiguousarray(
            Wk_l.reshape(256, NK, P).transpose(2, 1, 0).reshape(P, NK * 256))
        Wv_l = Wv[256 * h:256 * h + 256]
        wvpre = np.ascontiguousarray(
            Wv_l.reshape(256, NK, P).transpose(2, 1, 0).reshape(P, NK * 256))
        Wo_l = Wo[:, 1024 * h:1024 * h + 1024]                  # (2048, 1024)
        # wo_pre[m, p, kj*128 + jo] = Wo_l[128m+jo, 128kj+p]
        wopre = np.ascontiguousarray(
            Wo_l.reshape(16, P, 8, P).transpose(0, 3, 2, 1).reshape(16, P, 1024))
        bq_t = np.ascontiguousarray(
            bq[1024 * h:1024 * h + 1024].reshape(8, P).T)       # (128, 8)
        bk_t = np.ascontiguousarray(bk[256 * h:256 * h + 256].reshape(2, P).T)
        bv_rep = np.ascontiguousarray(
            np.broadcast_to(bv[256 * h:256 * h + 256][None, :], (P, 256)))
        in_maps.append({
            "xp": xpre, "wq": wqpre, "wk": wkpre, "wv": wvpre, "wo": wopre,
            "ct": ct, "mk": mask, "bqd": bq_t, "bkd": bk_t, "bvd": bv_rep,
            "oned": np.ones((P, P), np.float32),
        })
    return in_maps


def kernel(x, Wq, bq, Wk, bk, Wv, bv, Wo, bo):
    global _RUNNER
    args = [np.asarray(a, np.float32)
            for a in (x, Wq, bq, Wk, bk, Wv, bv, Wo, bo)]
    x, Wq, bq, Wk, bk, Wv, bv, Wo, bo = args
    if _RUNNER is None:
        _RUNNER = _build_runner()
    in_maps = _host_prep(x, Wq, bq, Wk, bk, Wv, bv, Wo, bo)
    res = _RUNNER(in_maps)
    outp = np.empty((B, T, N_EMBD), np.float32)
    for b in range(B):
        outp[b] = (res.results[2 * b]["out"] + res.results[2 * b + 1]["out"]).T
    outp += bo[None, None, :]
    return outp


# revision 18
# speedup vs baseline: 1.4296x; 1.4296x over previous
"""GQA attention kernel for 8 TRN2 NeuronCores.

Sharding: core c handles batch b=c//2 and head-half h=c%2 (8 q heads, 2 kv
heads per core).  Projections are column-parallel (q/k/v) and row-parallel
(out_proj); the host sums the two partial outputs per batch (no on-device
collectives).

The reference "rope" degenerates to an elementwise scale Y *= C with
C[t,j] = cos(t*inv[j%64]) + sin(t*inv[j%64]), folded into the q/k PSUM
eviction.  Softmax is computed without max-subtraction (scores are O(10),
exp is safe in f32): scores are built transposed (ki on partitions, qi on
free) so exp lands directly in the layout the y-matmul needs; the row sums
are accumulated with an all-ones lhsT matmul which also broadcasts them
across all 128 partitions for the final divide.
"""

import sys

if '/opt/trn_rl_repo' not in sys.path:
    sys.path.insert(0, '/opt/trn_rl_repo')

import numpy as np
import ml_dtypes

BF16 = ml_dtypes.bfloat16

N_EMBD = 2048
HD = 128          # head dim
T = 1024          # seq len
B = 4             # batch
NK = 16           # contraction tiles over n_embd
P = 128
F32 = None        # filled after mybir import
SCALE = 1.0 / np.sqrt(HD)

_RUNNER = None
_NC = None


def _build_runner():
    from concourse import bacc, tile, mybir
    from concourse.bass_utils import run_bass_kernel_spmd

    f32 = mybir.dt.float32
    f32r = mybir.dt.float32r
    bf16 = mybir.dt.bfloat16
    AF = mybir.ActivationFunctionType
    ALU = mybir.AluOpType

    nc = bacc.Bacc("TRN2", target_bir_lowering=False, debug=False, num_devices=8)

    xp = nc.dram_tensor("xp", [P, NK * T], bf16, kind="ExternalInput").ap()
    wq = nc.dram_tensor("wq", [8, P, 2048], bf16, kind="ExternalInput").ap()
    wk = nc.dram_tensor("wk", [P, NK * 256], bf16, kind="ExternalInput").ap()
    wv = nc.dram_tensor("wv", [P, NK * 256], bf16, kind="ExternalInput").ap()
    wo = nc.dram_tensor("wo", [16, P, 1024], bf16, kind="ExternalInput").ap()
    ct = nc.dram_tensor("ct", [P, T], f32, kind="ExternalInput").ap()
    mk = nc.dram_tensor("mk", [P, 2 * 256], f32, kind="ExternalInput").ap()
    bqd = nc.dram_tensor("bqd", [P, 8], f32, kind="ExternalInput").ap()
    bkd = nc.dram_tensor("bkd", [P, 2], f32, kind="ExternalInput").ap()
    bvd = nc.dram_tensor("bvd", [P, 256], f32, kind="ExternalInput").ap()
    oned = nc.dram_tensor("oned", [P, P], f32r, kind="ExternalInput").ap()
    out = nc.dram_tensor("out", [2048, T], f32, kind="ExternalOutput").ap()

    with tile.TileContext(nc) as tc:
        with (
            tc.tile_pool(name="const", bufs=1) as cpool,
            tc.tile_pool(name="qkv", bufs=1) as qkvpool,
        ):
            ct_sb = cpool.tile([P, T], f32, tag="ct")
            mk_sb = cpool.tile([P, 2 * 256], f32, tag="mk")
            bq_sb = cpool.tile([P, 8], f32, tag="bq")
            bk_sb = cpool.tile([P, 2], f32, tag="bk")
            bv_sb = cpool.tile([P, 256], f32, tag="bv")
            ones_sb = cpool.tile([P, P], f32r, tag="ones")

            qT = [qkvpool.tile([P, T], f32r, tag=f"qT{g}", name=f"qT{g}") for g in range(8)]
            kT = [qkvpool.tile([P, T], f32r, tag=f"kT{m}", name=f"kT{m}") for m in range(2)]
            vsb = [qkvpool.tile([P, 256], f32r, tag=f"v{tt}", name=f"v{tt}") for tt in range(8)]

            # ---------------- phase 1: projections ----------------
            with (
                tc.tile_pool(name="xt", bufs=16) as xpool,
                tc.tile_pool(name="wkv", bufs=32) as wkvpool,
                tc.tile_pool(name="wqs", bufs=3) as wqpool,
                tc.tile_pool(name="pp", bufs=8, space="PSUM") as pppool,
            ):
                x_sb = []
                wk_sb = []
                wv_sb = []
                for kc in range(NK):
                    xt = xpool.tile([P, T], bf16, tag="x")
                    nc.sync.dma_start(xt[:], xp[:, kc * T:(kc + 1) * T])
                    x_sb.append(xt)
                    wkt = wkvpool.tile([P, 256], bf16, tag="wk")
                    nc.sync.dma_start(wkt[:], wk[:, kc * 256:(kc + 1) * 256])
                    wk_sb.append(wkt)
                    wvt = wkvpool.tile([P, 256], bf16, tag="wv")
                    nc.sync.dma_start(wvt[:], wv[:, kc * 256:(kc + 1) * 256])
                    wv_sb.append(wvt)
                    if kc == 9:
                        nc.sync.dma_start(bk_sb[:], bkd[:])
                        nc.sync.dma_start(bv_sb[:], bvd[:])
                        nc.sync.dma_start(bq_sb[:], bqd[:])
                        nc.sync.dma_start(ct_sb[:], ct[:])
                        nc.sync.dma_start(ones_sb[:], oned[:])
                        nc.sync.dma_start(mk_sb[:], mk[:])

                # k projection: kT[m] (d on partitions, t free)
                for m in range(2):
                    for n in range(2):
                        ps = pppool.tile([P, 512], f32, tag="pp")
                        for kc in range(NK):
                            nc.tensor.matmul(
                                ps[:],
                                lhsT=wk_sb[kc][:, 128 * m:128 * m + 128],
                                rhs=x_sb[kc][:, 512 * n:512 * n + 512],
                                start=(kc == 0), stop=(kc == NK - 1),
                            )
                        nc.vector.scalar_tensor_tensor(
                            out=kT[m][:, 512 * n:512 * n + 512],
                            in0=ps[:], scalar=bk_sb[:, m:m + 1],
                            in1=ct_sb[:, 512 * n:512 * n + 512],
                            op0=ALU.add, op1=ALU.mult,
                        )

                # v projection: v (t on partitions, kv-dim free)
                for tt in range(8):
                    ps = pppool.tile([P, 256], f32, tag="pp")
                    for kc in range(NK):
                        nc.tensor.matmul(
                            ps[:],
                            lhsT=x_sb[kc][:, 128 * tt:128 * tt + 128],
                            rhs=wv_sb[kc][:],
                            start=(kc == 0), stop=(kc == NK - 1),
                        )
                    nc.vector.tensor_add(vsb[tt][:], ps[:], bv_sb[:])

                # q projection: qT[g] (d on partitions, t free)
                for g in range(8):
                    wqt = wqpool.tile([P, 2048], bf16, tag="wq")
                    nc.sync.dma_start(wqt[:], wq[g])
                    for n in range(2):
                        ps = pppool.tile([P, 512], f32, tag="pp")
                        for kc in range(NK):
                            nc.tensor.matmul(
                                ps[:],
                                lhsT=wqt[:, 128 * kc:128 * kc + 128],
                                rhs=x_sb[kc][:, 512 * n:512 * n + 512],
                                start=(kc == 0), stop=(kc == NK - 1),
                            )
                        nc.vector.scalar_tensor_tensor(
                            out=qT[g][:, 512 * n:512 * n + 512],
                            in0=ps[:], scalar=bq_sb[:, g:g + 1],
                            in1=ct_sb[:, 512 * n:512 * n + 512],
                            op0=ALU.add, op1=ALU.mult,
                        )

            # ---------------- phase 2+3: attention + out-proj ----------------
            with (
                tc.tile_pool(name="yT", bufs=1) as ypool,
                tc.tile_pool(name="exp", bufs=4) as epool,
                tc.tile_pool(name="rcp", bufs=2) as rpool,
                tc.tile_pool(name="wos", bufs=3) as wopool,
                tc.tile_pool(name="ost", bufs=4) as ostpool,
                tc.tile_pool(name="ps_s", bufs=2, space="PSUM") as spsum,
                tc.tile_pool(name="ps_y", bufs=1, space="PSUM") as ypsum,
                tc.tile_pool(name="ps_n", bufs=1, space="PSUM") as npsum,
                tc.tile_pool(name="ps_o", bufs=2, space="PSUM") as opsum,
            ):
                yT = [ypool.tile([P, T], bf16, tag=f"yT{g}", name=f"yT{g}") for g in range(8)]

                for c in range(4):
                    for g in range(8):
                        kg = g // 4
                        ps_y = ypsum.tile([P, 256], f32, tag="y")
                        ps_n = npsum.tile([P, 256], f32, tag="n")
                        R = 2 * c + 2
                        q_sl = qT[g][:, 256 * c:256 * c + 256]
                        # score blocks packed in groups of <=4 per 2-bank psum
                        # tile, one wide exp per pack
                        e_packs = []
                        for p0 in range(0, R, 4):
                            W = min(4, R - p0) * 256
                            ps_s = spsum.tile([P, 1024], f32, tag="s")
                            for j in range((W + 255) // 256):
                                nc.tensor.matmul(
                                    ps_s[:, 256 * j:256 * j + 256],
                                    lhsT=kT[kg][:, 128 * (p0 + j):128 * (p0 + j) + 128],
                                    rhs=q_sl,
                                    start=True, stop=True,
                                )
                            e = epool.tile([P, 1024], f32r, tag="e")
                            nc.scalar.activation(
                                e[:, 0:W], ps_s[:, 0:W], AF.Exp, scale=SCALE)
                            e_packs.append(e)
                        for rr in range(R):
                            e_sl = e_packs[rr // 4][:, 256 * (rr % 4):256 * (rr % 4) + 256]
                            if rr >= 2 * c:
                                i = rr - 2 * c
                                nc.vector.tensor_mul(
                                    e_sl, e_sl, mk_sb[:, 256 * i:256 * i + 256])
                            nc.tensor.matmul(
                                ps_y[:],
                                lhsT=vsb[rr][:, 128 * kg:128 * kg + 128],
                                rhs=e_sl,
                                start=(rr == 0), stop=(rr == R - 1),
                            )
                            nc.tensor.matmul(
                                ps_n[:],
                                lhsT=ones_sb[:],
                                rhs=e_sl,
                                start=(rr == 0), stop=(rr == R - 1),
                            )
                        rc = rpool.tile([P, 256], f32, tag="rc")
                        nc.vector.reciprocal(rc[:], ps_n[:])
                        nc.vector.tensor_mul(
                            yT[g][:, 256 * c:256 * c + 256], ps_y[:], rc[:])

                # out projection: outT (o on partitions, t free), partial sum
                for m in range(16):
                    wot = wopool.tile([P, 1024], bf16, tag="wo")
                    nc.sync.dma_start(wot[:], wo[m])
                    for n in range(2):
                        ps = opsum.tile([P, 512], f32, tag="o")
                        for kj in range(8):
                            nc.tensor.matmul(
                                ps[:],
                                lhsT=wot[:, 128 * kj:128 * kj + 128],
                                rhs=yT[kj][:, 512 * n:512 * n + 512],
                                start=(kj == 0), stop=(kj == 7),
                            )
                        ot = ostpool.tile([P, 512], f32, tag="ost")
                        nc.scalar.copy(ot[:], ps[:])
                        nc.sync.dma_start(
                            out[128 * m:128 * m + 128, 512 * n:512 * n + 512], ot[:])

    nc.compile()
    global _NC
    _NC = nc

    def run(in_maps, **kw):
        return run_bass_kernel_spmd(nc, in_maps, core_ids=list(range(8)), **kw)

    return run


def _host_prep(x, Wq, bq, Wk, bk, Wv, bv, Wo, bo):
    """Build the 8 per-core input maps."""
    inv = 10000.0 ** (-2.0 * np.arange(HD // 2) / HD)
    theta = np.arange(T)[:, None] * inv[None, :]
    C = np.concatenate([np.cos(theta) + np.sin(theta)] * 2, 1).astype(np.float32)
    ct = np.ascontiguousarray(C.T)                              # (128, 1024)

    mask = np.zeros((P, 2 * 256), np.float32)
    jj = np.arange(256)
    pp = np.arange(P)[:, None]
    for i in range(2):
        mask[:, 256 * i:256 * (i + 1)] = (jj[None, :] >= 128 * i + pp)

    in_maps = []
    for c in range(8):
        b, h = c // 2, c % 2
        xb = x[b]                                               # (t, 2048)
        # x_pre[p, kc*T + t] = x[b, t, 128*kc + p]
        xpre = np.ascontiguousarray(
            xb.reshape(T, NK, P).transpose(2, 1, 0).reshape(P, NK * T))
        Wq_l = Wq[1024 * h:1024 * h + 1024]
        # wq_pre[m, p, kc*128 + j] = Wq_l[128m+j, 128kc+p]
        wqpre = np.ascontiguousarray(
            Wq_l.reshape(8, P, NK, P).transpose(0, 3, 2, 1).reshape(8, P, 2048))
        Wk_l = Wk[256 * h:256 * h + 256]
        # wk_pre[p, kc*256 + j] = Wk_l[j, 128kc+p]
        wkpre = np.ascontiguousarray(
            Wk_l.reshape(256, NK, P).transpose(2, 1, 0).reshape(P, NK * 256))
        Wv_l = Wv[256 * h:256 * h + 256]
        wvpre = np.ascontiguousarray(
            Wv_l.reshape(256, NK, P).transpose(2, 1, 0).reshape(P, NK * 256))
        Wo_l = Wo[:, 1024 * h:1024 * h + 1024]                  # (2048, 1024)
        # wo_pre[m, p, kj*128 + jo] = Wo_l[128m+jo, 128kj+p]
        wopre = np.ascontiguousarray(
            Wo_l.reshape(16, P, 8, P).transpose(0, 3, 2, 1).reshape(16, P, 1024))
        bq_t = np.ascontiguousarray(
            bq[1024 * h:1024 * h + 1024].reshape(8, P).T)       # (128, 8)
        bk_t = np.ascontiguousarray(bk[256 * h:256 * h + 256].reshape(2, P).T)
        bv_rep = np.ascontiguousarray(
            np.broadcast_to(bv[256 * h:256 * h + 256][None, :], (P, 256)))
        in_maps.append({
            "xp": xpre.astype(BF16), "wq": wqpre.astype(BF16),
            "wk": wkpre.astype(BF16), "wv": wvpre.astype(BF16),
            "wo": wopre.astype(BF16),
            "ct": ct, "mk": mask, "bqd": bq_t, "bkd": bk_t, "bvd": bv_rep,
            "oned": np.ones((P, P), np.float32),
        })
    return in_maps


def kernel(x, Wq, bq, Wk, bk, Wv, bv, Wo, bo):
    global _RUNNER
    args = [np.asarray(a, np.float32)
            for a in (x, Wq, bq, Wk, bk, Wv, bv, Wo, bo)]
    x, Wq, bq, Wk, bk, Wv, bv, Wo, bo = args
    if _RUNNER is None:
        _RUNNER = _build_runner()
    in_maps = _host_prep(x, Wq, bq, Wk, bk, Wv, bv, Wo, bo)
    res = _RUNNER(in_maps)
    outp = np.empty((B, T, N_EMBD), np.float32)
    for b in range(B):
        outp[b] = (res.results[2 * b]["out"] + res.results[2 * b + 1]["out"]).T
    outp += bo[None, None, :]
    return outp
